# revision 12
# baseline (speedup 1.0000x reference)
"""BinaryResNetE18 forward on 8 TRN2 NeuronCores (pure data parallel).

- 32 images -> 8 cores x 4 images, no collectives.
- Device computes in "t-space": t = h - B (B = accumulated per-channel
  constant): BN+residual = one DVE op, next-block sign = one ACT op.
- Binary convs exact in bf16 (sign x sign, fp32 PSUM).
- Real-valued path (stem, shortcuts, BN, head) fp32-exact; stem conv uses
  bf16 hi/lo 3-term split (error ~2^-18; the net is chaotic so bf16 on the
  real path is catastrophic, but 2^-18 is below the flip threshold).
- Stem input: host passes x as zero-padded CHW bf16 hi/lo planes (pure
  layout/dtype transform).  Device replicates rows into a 42-partition
  (c,kw,kh-parity) tap tensor with contiguous-run DMAs; 4 kh-rounds x 3
  terms of K=42 matmuls accumulate in PSUM.
"""

import contextlib
import numpy as np
import ml_dtypes

import concourse.bass as bass
import concourse.mybir as mybir
import concourse.tile as tile
from concourse import bacc
from concourse.bass_utils import run_bass_kernel_spmd

F32 = mybir.dt.float32
BF16 = mybir.dt.bfloat16
F8 = mybir.dt.float8e4
AF = mybir.ActivationFunctionType
ALU = mybir.AluOpType
AX = mybir.AxisListType

EPS = 1e-5
NB = 4
NCORES = 8
HP, WP = 231, 236      # padded CHW x: rows -2..228, cols -2..233
OH = 112

BLOCKS = []
_c = 64
for _f in [64, 128, 256, 512]:
    for _ in range(4):
        BLOCKS.append((_c, _f, 2 if _c != _f else 1, _c != _f))
        _c = _f
H_IN = []
_h = 56
for (_ci, _co, _s, _dn) in BLOCKS:
    H_IN.append(_h)
    if _s == 2:
        _h //= 2

SEC_GEOM = {1: (64, 1, 56), 2: (128, 1, 28), 3: (128, 2, 14), 4: (128, 4, 7)}


def sec_of(i):
    """Section of block i's INPUT grid (block 16 == head)."""
    return 1 + sum(i > j for j in (4, 8, 12))


# ---------------------------------------------------------------------------
# host-side folding
# ---------------------------------------------------------------------------

def _bn_fold(p):
    s = np.float64(np.asarray(p['gamma'])) / np.sqrt(np.float64(np.asarray(p['var'])) + EPS)
    b = np.float64(np.asarray(p['beta'])) - np.float64(np.asarray(p['mean'])) * s
    return s, b


def _hi_lo(a):
    hi = np.asarray(a, np.float32).astype(ml_dtypes.bfloat16)
    lo = (np.asarray(a, np.float32) - hi.astype(np.float32)).astype(ml_dtypes.bfloat16)
    return hi, lo


def prep_params(params):
    out = {}
    s1, b1 = _bn_fold(params['stem_bn1'])
    s2, b2 = _bn_fold(params['stem_bn2'])
    out['stem_scale'] = (s1 * s2).astype(np.float32)
    out['stem_bias'] = (b1 * s2).astype(np.float32)
    wst = np.asarray(params['stem_w'], np.float32)        # [7,7,3,64]
    w42 = np.zeros((4, 42, 64), np.float32)
    for d in range(4):
        for c in range(3):
            for kw in range(7):
                for kap in range(2):
                    kh = 2 * d + kap
                    if kh < 7:
                        w42[d, c * 14 + kap * 7 + kw] = wst[kh, kw, c]
    out['wst_hi'], out['wst_lo'] = _hi_lo(w42)
    B = b2.copy()
    out['B_stem'] = B.astype(np.float32)
    for i, bp in enumerate(params['blocks']):
        ci, co, stride, down = BLOCKS[i]
        w = np.asarray(bp['w'], np.float32)
        out[f'wb{i}'] = np.ascontiguousarray(
            np.where(w >= 0, np.float32(1), np.float32(-1))
            .reshape(9, ci, co)).astype(np.dtype(ml_dtypes.float8_e4m3))
        sb, bb = _bn_fold(bp['bn'])
        if down:
            wd = np.asarray(bp['down_w'], np.float32)[0, 0]
            sd, bd = _bn_fold(bp['down_bn'])
            cd = B @ np.float64(wd)
            out[f'wd{i}'] = (wd / 4.0).astype(np.float32)
            out[f'sd{i}'] = sd.astype(np.float32)
            B = bb + cd * sd + bd
        else:
            B = B + bb
        out[f'sb{i}'] = sb.astype(np.float32)
        out[f'B{i}'] = B.astype(np.float32)
    out['whead'] = np.ascontiguousarray(np.asarray(params['head_w'], np.float32))
    out['bhead'] = np.asarray(params['head_b'], np.float32)
    return out


def _pack_consts(pp):
    cols, colmap = [], {}

    def add(name, vec):
        v = np.asarray(vec, np.float32).reshape(-1)
        g = int(np.ceil(len(v) / 128))
        colmap[name] = len(cols)
        for j in range(g):
            c = np.zeros(128, np.float32)
            ch = v[j * 128:(j + 1) * 128]
            c[:len(ch)] = ch
            cols.append(c)

    add('stem_scale', pp['stem_scale'])
    add('stem_bias', pp['stem_bias'])
    add('B_stem', pp['B_stem'])
    for i in range(16):
        add(f'sb{i}', pp[f'sb{i}'])
        add(f'B{i}', pp[f'B{i}'])
        if f'sd{i}' in pp:
            add(f'sd{i}', pp[f'sd{i}'])
    return np.ascontiguousarray(np.stack(cols, axis=1)), colmap


def prep_x_core(xc):
    xt = np.transpose(np.asarray(xc, np.float32), (0, 3, 1, 2))
    xp = np.zeros((NB, 3, HP, WP), np.float32)
    xp[:, :, 2:226, 2:226] = xt
    return _hi_lo(xp)


# ---------------------------------------------------------------------------
# device body
# ---------------------------------------------------------------------------

def _chunks(Ho):
    if Ho == 56:
        return [(i, r, r + 8) for i in range(NB) for r in range(0, 56, 8)]
    if Ho == 28:
        return [(i, r, min(r + 16, 28)) for i in range(NB) for r in range(0, 28, 16)]
    if Ho == 14:
        return [(None, 0, 7), (None, 7, 14)]
    if Ho == 7:
        return [(None, 0, 7)]
    raise ValueError(Ho)


def emit_model(tc, io, colmap):
    nc = tc.nc

    with contextlib.ExitStack() as top:
        g_pool = top.enter_context(tc.tile_pool(name="g", bufs=1))
        psum_pool = top.enter_context(tc.tile_pool(name="psum", bufs=1, space="PSUM"))
        w_pool = top.enter_context(tc.tile_pool(name="w", bufs=2))
        d_pool = top.enter_context(tc.tile_pool(name="d", bufs=2))

        NC = io['consts'].shape[1]
        ct = g_pool.tile([128, NC], F32)
        nc.sync.dma_start(out=ct[:, :], in_=io['consts'][:, :])

        def cvec(name, g=0, rows=128):
            return ct[:rows, colmap[name] + g:colmap[name] + g + 1]

        wst_hi = g_pool.tile([42, 4, 64], BF16)
        wst_lo = g_pool.tile([42, 4, 64], BF16)
        for t, nm in ((wst_hi, 'wst_hi'), (wst_lo, 'wst_lo')):
            src = bass.AP(tensor=io[nm].tensor, offset=0,
                          ap=[[64, 42], [42 * 64, 4], [1, 64]])
            nc.sync.dma_start(out=t[:, :, :], in_=src)

        wdt = {}
        for i in (4, 8, 12):
            ci, co, _, _ = BLOCKS[i]
            gi, k = max(ci // 128, 1), min(ci, 128)
            t = g_pool.tile([k, gi, co], F32, name=f"wd{i}_t")
            src = bass.AP(tensor=io[f'wd{i}'].tensor, offset=0,
                          ap=[[co, k], [128 * co, gi], [1, co]])
            nc.sync.dma_start(out=t[:, :, :], in_=src)
            wdt[i] = t

        # section pools opened lazily, closed when dead
        sec_cm, sec_pool, t_tiles, s_tiles = {}, {}, {}, {}

        def open_sec(s):
            if s in sec_pool:
                return
            sec_cm[s] = tc.tile_pool(name=f"sec{s}", bufs=1)
            p = sec_cm[s].__enter__()
            sec_pool[s] = p
            P, G, H = SEC_GEOM[s]
            t_tiles[s] = p.tile([P, G, NB, H, H], F32, tag=f"t{s}", name=f"t{s}")
            s_tiles[s] = {}

        def get_s(s, k):
            if k not in s_tiles[s]:
                P, G, H = SEC_GEOM[s]
                sp = sec_pool[s].tile([P, G, NB, H + 2, H + 2], F8,
                                      tag=f"s{s}_{k}", name=f"s{s}_{k}")
                nc.vector.memset(sp[:, :, :, :, :], 0.0)
                s_tiles[s][k] = sp
            return s_tiles[s][k]

        def close_sec(s):
            sec_cm[s].__exit__(None, None, None)

        # ================= STEM =================
        open_sec(1)
        t1, s1 = t_tiles[1], get_s(1, 0)
        with tc.tile_pool(name="stem", bufs=1) as stp, \
                tc.tile_pool(name="stm2", bufs=2) as stp2:
            sb_ap = cvec('stem_bias', rows=64)
            ss_ap = cvec('stem_scale', rows=64)
            for img in range(NB):
                m1 = stp.tile([64, 112, 56], F32, tag="m1", name="m1")
                for slab in range(14):
                    o0 = slab * 8
                    xwh = stp.tile([42, 21, WP], BF16, tag="xwh", name="xwh")
                    xwl = stp.tile([42, 21, WP], BF16, tag="xwl", name="xwl")
                    for xt, nm in ((xwh, 'x_hi'), (xwl, 'x_lo')):
                        for c in range(3):
                            for kap in range(2):
                                sap = bass.AP(
                                    tensor=io[nm].tensor,
                                    offset=(img * 3 + c) * HP * WP
                                    + (2 * o0 + kap) * WP,
                                    ap=[[1, 7], [1, 21 * WP]])
                                nc.sync.dma_start(
                                    out=xt[c * 14 + kap * 7:c * 14 + kap * 7 + 7, :, :],
                                    in_=sap)
                    for c0 in range(2):
                        ps = psum_pool.tile([64, 4, OH], F32, tag=f"ps{c0}", name="ps")
                        first = True
                        for d in range(4):
                            r = 8 * c0 + 2 * d
                            for ti, (xt, wt0) in enumerate(
                                    ((xwh, wst_hi), (xwh, wst_lo), (xwl, wst_hi))):
                                rhs = xt[:, r:r + 7:2, 0:2 * OH:2]
                                nc.tensor.matmul(ps[:, :, :], lhsT=wt0[:, d, :],
                                                 rhs=rhs, start=first,
                                                 stop=(d == 3 and ti == 2))
                                first = False
                        act = stp2.tile([64, 4, OH], F32, tag="act", name="act")
                        nc.scalar.activation(act[:, :, :], ps[:, :, :], AF.Relu,
                                             bias=sb_ap, scale=ss_ap)
                        r0 = o0 + c0 * 4
                        nc.vector.tensor_max(m1[:, r0:r0 + 4, :],
                                             act[:, :, 0:112:2], act[:, :, 1:112:2])
                        nc.vector.tensor_max(m1[:, r0:r0 + 4, 0:55],
                                             m1[:, r0:r0 + 4, 0:55],
                                             act[:, :, 2:112:2])
                ts = t1[:, 0, img]
                nc.vector.tensor_max(ts[:, :, :], m1[:, 0:112:2, :], m1[:, 1:112:2, :])
                nc.vector.tensor_max(ts[:, 0:55, :], ts[:, 0:55, :], m1[:, 2:112:2, :])
                nc.scalar.activation(s1[:, 0, img, 1:57, 1:57], ts[:, :, :],
                                     AF.Sign, bias=cvec('B_stem', rows=64))

        # ================= blocks =================
        for i in range(16):
            ci, co, stride, down = BLOCKS[i]
            si, so = sec_of(i), sec_of(i + 1)
            open_sec(so)
            Hi, Ho = H_IN[i], H_IN[i] // stride
            Gi, Go = max(ci // 128, 1), max(co // 128, 1)
            K, M = min(ci, 128), min(co, 128)
            s_in = get_s(si, i % 2)
            t_in = t_tiles[si]
            t_out = t_tiles[so]
            s_out = get_s(so, (i + 1) % 2) if i < 15 else None

            for go in range(Go):
                wt = w_pool.tile([K, 9, Gi, M], F8, tag="wb", name="wb")
                src = bass.AP(tensor=io[f'wb{i}'].tensor, offset=go * M,
                              ap=[[co, K], [ci * co, 9], [128 * co, Gi], [1, M]])
                nc.sync.dma_start(out=wt[:, :, :, :], in_=src)
                sb_i = cvec(f'sb{i}', go, rows=M)
                B_i = cvec(f'B{i}', go, rows=M) if i < 15 else None
                allc = _chunks(Ho)
                CG = 4 if len(allc) >= 4 else len(allc)
                for cg0 in range(0, len(allc), CG):
                    grp = allc[cg0:cg0 + CG]
                    pss, psfs = [], []
                    for (img, r0, r1) in grp:
                        nr = r1 - r0
                        pshape = [M, NB, nr, Ho] if img is None else [M, nr, Ho]
                        ps = psum_pool.tile(pshape, F32, tag=f"ps{len(pss)}",
                                            name="ps")
                        pss.append(ps)
                        psfs.append(ps[:, :, :, :] if img is None else ps[:, :, :])
                    k_mm = 0
                    for gi in range(Gi):
                        for tap in range(9):
                            kh, kw = tap // 3, tap % 3
                            k_mm += 1
                            for ci_, (img, r0, r1) in enumerate(grp):
                                if stride == 1:
                                    rhs = (s_in[:, gi, :, r0 + kh:r1 + kh, kw:kw + Ho]
                                           if img is None else
                                           s_in[:, gi, img, r0 + kh:r1 + kh, kw:kw + Ho])
                                else:
                                    rhs = (s_in[:, gi, :, 2 * r0 + kh + 1:2 * r1 + kh:2,
                                                kw + 1:kw + 2 * Ho:2]
                                           if img is None else
                                           s_in[:, gi, img, 2 * r0 + kh + 1:2 * r1 + kh:2,
                                                kw + 1:kw + 2 * Ho:2])
                                nc.tensor.matmul(psfs[ci_], lhsT=wt[:, tap, gi, :],
                                                 rhs=rhs, start=(k_mm == 1),
                                                 stop=(k_mm == Gi * 9))
                    for ci_, (img, r0, r1) in enumerate(grp):
                        nr = r1 - r0
                        ps, psf = pss[ci_], psfs[ci_]

                        pshape = [M, NB, nr, Ho] if img is None else [M, nr, Ho]

                        def tsl(tt, g):
                            return (tt[:, g, :, r0:r1, :] if img is None
                                    else tt[:, g, img, r0:r1, :])

                        if not down:
                            tin = tsl(t_in, go)
                            nc.vector.scalar_tensor_tensor(
                                out=tin, in0=psf, scalar=sb_i, in1=tin,
                                op0=ALU.mult, op1=ALU.add)
                            tview = tin
                        else:
                            ps2 = psum_pool.tile(pshape, F32, tag="psd", name="ps2")
                            ps2f = ps2[:, :, :, :] if img is None else ps2[:, :, :]
                            f2 = True
                            for gi in range(Gi):
                                for (dh, dw) in ((0, 0), (0, 1), (1, 0), (1, 1)):
                                    rhs = (t_in[:, gi, :, 2 * r0 + dh:2 * r1 + dh - 1:2,
                                                dw:dw + 2 * Ho - 1:2]
                                           if img is None else
                                           t_in[:, gi, img, 2 * r0 + dh:2 * r1 + dh - 1:2,
                                                dw:dw + 2 * Ho - 1:2])
                                    nc.tensor.matmul(
                                        ps2f, lhsT=wdt[i][:, gi, go * M:(go + 1) * M],
                                        rhs=rhs, start=f2,
                                        stop=(gi == Gi - 1 and (dh, dw) == (1, 1)))
                                    f2 = False
                            tmp = d_pool.tile(pshape, F32, tag="dtmp", name="dtmp")
                            tmpf = tmp[:, :, :, :] if img is None else tmp[:, :, :]
                            nc.vector.tensor_scalar_mul(tmpf, ps2f, cvec(f'sd{i}', go, rows=M))
                            tout = tsl(t_out, go)
                            nc.vector.scalar_tensor_tensor(
                                out=tout, in0=psf, scalar=sb_i, in1=tmpf,
                                op0=ALU.mult, op1=ALU.add)
                            tview = tout
                        if i < 15:
                            ssl = (s_out[:, go, :, 1 + r0:1 + r1, 1:1 + Ho] if img is None
                                   else s_out[:, go, img, 1 + r0:1 + r1, 1:1 + Ho])
                            nc.scalar.activation(ssl, tview, AF.Sign, bias=B_i)

        # ================= head =================
        h_cm = tc.tile_pool(name="head", bufs=1)
        g_pool = h_cm.__enter__()
        hw_t = g_pool.tile([128, 4, 1000], F32)
        nc.sync.dma_start(out=hw_t[:, :, :],
                          in_=bass.AP(tensor=io['whead'].tensor, offset=0,
                                      ap=[[1000, 128], [128 * 1000, 4], [1, 1000]]))
        hb_t = g_pool.tile([NB, 1000], F32)
        nc.sync.dma_start(out=hb_t[:, :],
                          in_=bass.AP(tensor=io['bhead'].tensor, offset=0,
                                      ap=[[0, NB], [1, 1000]]))
        t4 = t_tiles[4]
        u = g_pool.tile([128, 4, NB, 7, 7], F32)
        g_t = g_pool.tile([128, 4, NB], F32)
        for g in range(4):
            nc.scalar.activation(u[:, g, :, :, :], t4[:, g, :, :, :], AF.Relu,
                                 bias=cvec('B15', g))
            nc.vector.tensor_reduce(g_t[:, g, :], u[:, g, :, :, :],
                                    axis=AX.XY, op=ALU.add)
        lg = g_pool.tile([NB, 1000], F32)
        for nt in range(2):
            lps = psum_pool.tile([NB, 500], F32, tag=f"ps{nt}", name="lps")
            for g in range(4):
                nc.tensor.matmul(lps[:, :], lhsT=g_t[:, g, :],
                                 rhs=hw_t[:, g, nt * 500:(nt + 1) * 500],
                                 start=(g == 0), stop=(g == 3))
            nc.vector.scalar_tensor_tensor(
                out=lg[:, nt * 500:(nt + 1) * 500], in0=lps[:, :],
                scalar=1.0 / 49.0, in1=hb_t[:, nt * 500:(nt + 1) * 500],
                op0=ALU.mult, op1=ALU.add)
        mx = g_pool.tile([NB, 1], F32)
        nc.vector.tensor_reduce(mx[:, :], lg[:, :], axis=AX.X, op=ALU.max)
        nmx = g_pool.tile([NB, 1], F32)
        nc.vector.tensor_scalar_mul(nmx[:, :], mx[:, :], -1.0)
        e = g_pool.tile([NB, 1000], F32)
        sm = g_pool.tile([NB, 1], F32)
        nc.scalar.activation(e[:, :], lg[:, :], AF.Exp, bias=nmx[:, :],
                             accum_out=sm[:, :])
        rs = g_pool.tile([NB, 1], F32)
        nc.vector.reciprocal(rs[:, :], sm[:, :])
        o = g_pool.tile([NB, 1000], F32)
        nc.vector.tensor_scalar_mul(o[:, :], e[:, :], rs[:, :])
        nc.sync.dma_start(out=io['out'][:, :], in_=o[:, :])
        h_cm.__exit__(None, None, None)
        for s in (4, 3, 2, 1):
            close_sec(s)


# ---------------------------------------------------------------------------
# build + run
# ---------------------------------------------------------------------------

_CACHE = {}


def declare_io(nc, nc_shape):
    io = {}
    io['x_hi'] = nc.dram_tensor("x_hi", [NB, 3, HP, WP], BF16, kind="ExternalInput").ap()
    io['x_lo'] = nc.dram_tensor("x_lo", [NB, 3, HP, WP], BF16, kind="ExternalInput").ap()
    io['wst_hi'] = nc.dram_tensor("wst_hi", [4, 42, 64], BF16, kind="ExternalInput").ap()
    io['wst_lo'] = nc.dram_tensor("wst_lo", [4, 42, 64], BF16, kind="ExternalInput").ap()
    for i in range(16):
        ci, co, _, dn = BLOCKS[i]
        io[f'wb{i}'] = nc.dram_tensor(f"wb{i}", [9, ci, co], F8, kind="ExternalInput").ap()
        if dn:
            io[f'wd{i}'] = nc.dram_tensor(f"wd{i}", [ci, co], F32, kind="ExternalInput").ap()
    io['consts'] = nc.dram_tensor("consts", list(nc_shape), F32, kind="ExternalInput").ap()
    io['whead'] = nc.dram_tensor("whead", [512, 1000], F32, kind="ExternalInput").ap()
    io['bhead'] = nc.dram_tensor("bhead", [1000], F32, kind="ExternalInput").ap()
    io['out'] = nc.dram_tensor("out", [NB, 1000], F32, kind="ExternalOutput").ap()
    return io


def build(consts_shape, colmap):
    nc = bacc.Bacc("TRN2", target_bir_lowering=False, debug=False,
                   enable_asserts=False)
    io = declare_io(nc, consts_shape)
    with tile.TileContext(nc) as tc:
        emit_model(tc, io, colmap)
    nc.compile()
    return nc, io


def make_in_maps(x, pp, consts_arr):
    base = {'wst_hi': pp['wst_hi'], 'wst_lo': pp['wst_lo'], 'consts': consts_arr,
            'whead': pp['whead'], 'bhead': pp['bhead']}
    for i in range(16):
        base[f'wb{i}'] = pp[f'wb{i}']
        if f'wd{i}' in pp:
            base[f'wd{i}'] = pp[f'wd{i}']
    in_maps = []
    for c in range(NCORES):
        xh, xl = prep_x_core(np.asarray(x)[c * NB:(c + 1) * NB])
        in_maps.append({**base, 'x_hi': xh, 'x_lo': xl})
    return in_maps


def kernel(x, params):
    x = np.asarray(x, np.float32)
    pp = prep_params(params)
    consts_arr, colmap = _pack_consts(pp)
    if 'nc' not in _CACHE:
        _CACHE['nc'] = build(consts_arr.shape, colmap)
    nc, io = _CACHE['nc']
    res = run_bass_kernel_spmd(nc, make_in_maps(x, pp, consts_arr),
                               list(range(NCORES)))
    return np.concatenate([r['out'] for r in res.results], axis=0)


# revision 13
# speedup vs baseline: 1.0142x; 1.0142x over previous
"""BinaryResNetE18 forward on 8 TRN2 NeuronCores (pure data parallel).

- 32 images -> 8 cores x 4 images, no collectives.
- Device computes in "t-space": t = h - B (B = accumulated per-channel
  constant): BN+residual = one DVE op, next-block sign = one ACT op.
- Binary convs exact in bf16 (sign x sign, fp32 PSUM).
- Real-valued path (stem, shortcuts, BN, head) fp32-exact; stem conv uses
  bf16 hi/lo 3-term split (error ~2^-18; the net is chaotic so bf16 on the
  real path is catastrophic, but 2^-18 is below the flip threshold).
- Stem input: host passes x as zero-padded CHW bf16 hi/lo planes (pure
  layout/dtype transform).  Device replicates rows into a 42-partition
  (c,kw,kh-parity) tap tensor with contiguous-run DMAs; 4 kh-rounds x 3
  terms of K=42 matmuls accumulate in PSUM.
"""

import contextlib
import numpy as np
import ml_dtypes

import concourse.bass as bass
import concourse.mybir as mybir
import concourse.tile as tile
from concourse import bacc
from concourse.bass_utils import run_bass_kernel_spmd

F32 = mybir.dt.float32
BF16 = mybir.dt.bfloat16
F8 = mybir.dt.float8e4
AF = mybir.ActivationFunctionType
ALU = mybir.AluOpType
AX = mybir.AxisListType

EPS = 1e-5
NB = 4
NCORES = 8
HP, WP = 231, 236      # padded CHW x: rows -2..228, cols -2..233
OH = 112

BLOCKS = []
_c = 64
for _f in [64, 128, 256, 512]:
    for _ in range(4):
        BLOCKS.append((_c, _f, 2 if _c != _f else 1, _c != _f))
        _c = _f
H_IN = []
_h = 56
for (_ci, _co, _s, _dn) in BLOCKS:
    H_IN.append(_h)
    if _s == 2:
        _h //= 2

SEC_GEOM = {1: (64, 1, 56), 2: (128, 1, 28), 3: (128, 2, 14), 4: (128, 4, 7)}


def sec_of(i):
    """Section of block i's INPUT grid (block 16 == head)."""
    return 1 + sum(i > j for j in (4, 8, 12))


# ---------------------------------------------------------------------------
# host-side folding
# ---------------------------------------------------------------------------

def _bn_fold(p):
    s = np.float64(np.asarray(p['gamma'])) / np.sqrt(np.float64(np.asarray(p['var'])) + EPS)
    b = np.float64(np.asarray(p['beta'])) - np.float64(np.asarray(p['mean'])) * s
    return s, b


def _hi_lo(a):
    hi = np.asarray(a, np.float32).astype(ml_dtypes.bfloat16)
    lo = (np.asarray(a, np.float32) - hi.astype(np.float32)).astype(ml_dtypes.bfloat16)
    return hi, lo


def prep_params(params):
    out = {}
    s1, b1 = _bn_fold(params['stem_bn1'])
    s2, b2 = _bn_fold(params['stem_bn2'])
    out['stem_scale'] = (s1 * s2).astype(np.float32)
    out['stem_bias'] = (b1 * s2).astype(np.float32)
    wst = np.asarray(params['stem_w'], np.float32)        # [7,7,3,64]
    w42 = np.zeros((4, 42, 64), np.float32)
    for d in range(4):
        for c in range(3):
            for kw in range(7):
                for kap in range(2):
                    kh = 2 * d + kap
                    if kh < 7:
                        w42[d, c * 14 + kap * 7 + kw] = wst[kh, kw, c]
    out['wst_hi'], out['wst_lo'] = _hi_lo(w42)
    B = b2.copy()
    out['B_stem'] = B.astype(np.float32)
    for i, bp in enumerate(params['blocks']):
        ci, co, stride, down = BLOCKS[i]
        w = np.asarray(bp['w'], np.float32)
        out[f'wb{i}'] = np.ascontiguousarray(
            np.where(w >= 0, np.float32(1), np.float32(-1))
            .reshape(9, ci, co)).astype(np.dtype(ml_dtypes.float8_e4m3))
        sb, bb = _bn_fold(bp['bn'])
        if down:
            wd = np.asarray(bp['down_w'], np.float32)[0, 0]
            sd, bd = _bn_fold(bp['down_bn'])
            cd = B @ np.float64(wd)
            out[f'wd{i}'] = (wd / 4.0).astype(np.float32)
            out[f'sd{i}'] = sd.astype(np.float32)
            B = bb + cd * sd + bd
        else:
            B = B + bb
        out[f'sb{i}'] = sb.astype(np.float32)
        out[f'B{i}'] = B.astype(np.float32)
    out['whead'] = np.ascontiguousarray(np.asarray(params['head_w'], np.float32))
    out['bhead'] = np.asarray(params['head_b'], np.float32)
    return out


def _pack_consts(pp):
    cols, colmap = [], {}

    def add(name, vec):
        v = np.asarray(vec, np.float32).reshape(-1)
        g = int(np.ceil(len(v) / 128))
        colmap[name] = len(cols)
        for j in range(g):
            c = np.zeros(128, np.float32)
            ch = v[j * 128:(j + 1) * 128]
            c[:len(ch)] = ch
            cols.append(c)

    add('stem_scale', pp['stem_scale'])
    add('stem_bias', pp['stem_bias'])
    add('B_stem', pp['B_stem'])
    for i in range(16):
        add(f'sb{i}', pp[f'sb{i}'])
        add(f'B{i}', pp[f'B{i}'])
        if f'sd{i}' in pp:
            add(f'sd{i}', pp[f'sd{i}'])
    return np.ascontiguousarray(np.stack(cols, axis=1)), colmap


def prep_x_core(xc):
    xt = np.transpose(np.asarray(xc, np.float32), (0, 3, 1, 2))
    xp = np.zeros((NB, 3, HP, WP), np.float32)
    xp[:, :, 2:226, 2:226] = xt
    return _hi_lo(xp)


# ---------------------------------------------------------------------------
# device body
# ---------------------------------------------------------------------------

def _chunks(Ho):
    if Ho == 56:
        return [(i, r, r + 8) for i in range(NB) for r in range(0, 56, 8)]
    if Ho == 28:
        return [(i, r, min(r + 16, 28)) for i in range(NB) for r in range(0, 28, 16)]
    if Ho == 14:
        return [(None, 0, 7), (None, 7, 14)]
    if Ho == 7:
        return [(None, 0, 7)]
    raise ValueError(Ho)


def emit_model(tc, io, colmap):
    nc = tc.nc

    with contextlib.ExitStack() as top:
        g_pool = top.enter_context(tc.tile_pool(name="g", bufs=1))
        psum_pool = top.enter_context(tc.tile_pool(name="psum", bufs=2, space="PSUM"))
        w_pool = top.enter_context(tc.tile_pool(name="w", bufs=2))
        d_pool = top.enter_context(tc.tile_pool(name="d", bufs=2))

        NC = io['consts'].shape[1]
        ct = g_pool.tile([128, NC], F32)
        nc.sync.dma_start(out=ct[:, :], in_=io['consts'][:, :])

        def cvec(name, g=0, rows=128):
            return ct[:rows, colmap[name] + g:colmap[name] + g + 1]

        wst_hi = g_pool.tile([42, 4, 64], BF16)
        wst_lo = g_pool.tile([42, 4, 64], BF16)
        for t, nm in ((wst_hi, 'wst_hi'), (wst_lo, 'wst_lo')):
            src = bass.AP(tensor=io[nm].tensor, offset=0,
                          ap=[[64, 42], [42 * 64, 4], [1, 64]])
            nc.sync.dma_start(out=t[:, :, :], in_=src)

        wdt = {}
        for i in (4, 8, 12):
            ci, co, _, _ = BLOCKS[i]
            gi, k = max(ci // 128, 1), min(ci, 128)
            t = g_pool.tile([k, gi, co], F32, name=f"wd{i}_t")
            src = bass.AP(tensor=io[f'wd{i}'].tensor, offset=0,
                          ap=[[co, k], [128 * co, gi], [1, co]])
            nc.sync.dma_start(out=t[:, :, :], in_=src)
            wdt[i] = t

        # section pools opened lazily, closed when dead
        sec_cm, sec_pool, t_tiles, s_tiles = {}, {}, {}, {}

        def open_sec(s):
            if s in sec_pool:
                return
            sec_cm[s] = tc.tile_pool(name=f"sec{s}", bufs=1)
            p = sec_cm[s].__enter__()
            sec_pool[s] = p
            P, G, H = SEC_GEOM[s]
            t_tiles[s] = p.tile([P, G, NB, H, H], F32, tag=f"t{s}", name=f"t{s}")
            s_tiles[s] = {}

        def get_s(s, k):
            if k not in s_tiles[s]:
                P, G, H = SEC_GEOM[s]
                sp = sec_pool[s].tile([P, G, NB, H + 2, H + 2], F8,
                                      tag=f"s{s}_{k}", name=f"s{s}_{k}")
                nc.vector.memset(sp[:, :, :, :, :], 0.0)
                s_tiles[s][k] = sp
            return s_tiles[s][k]

        def close_sec(s):
            sec_cm[s].__exit__(None, None, None)

        # ================= STEM =================
        open_sec(1)
        t1, s1 = t_tiles[1], get_s(1, 0)
        with tc.tile_pool(name="stem", bufs=1) as stp, \
                tc.tile_pool(name="stm2", bufs=2) as stp2:
            sb_ap = cvec('stem_bias', rows=64)
            ss_ap = cvec('stem_scale', rows=64)
            for img in range(NB):
                m1 = stp.tile([64, 112, 56], F32, tag="m1", name="m1")
                for slab in range(14):
                    o0 = slab * 8
                    xwh = stp.tile([42, 21, WP], BF16, tag="xwh", name="xwh")
                    xwl = stp.tile([42, 21, WP], BF16, tag="xwl", name="xwl")
                    for xt, nm in ((xwh, 'x_hi'), (xwl, 'x_lo')):
                        for c in range(3):
                            for kap in range(2):
                                sap = bass.AP(
                                    tensor=io[nm].tensor,
                                    offset=(img * 3 + c) * HP * WP
                                    + (2 * o0 + kap) * WP,
                                    ap=[[1, 7], [1, 21 * WP]])
                                nc.sync.dma_start(
                                    out=xt[c * 14 + kap * 7:c * 14 + kap * 7 + 7, :, :],
                                    in_=sap)
                    for c0 in range(2):
                        ps = psum_pool.tile([64, 4, OH], F32, tag=f"ps{c0}", name="ps")
                        first = True
                        for d in range(4):
                            r = 8 * c0 + 2 * d
                            for ti, (xt, wt0) in enumerate(
                                    ((xwh, wst_hi), (xwh, wst_lo), (xwl, wst_hi))):
                                rhs = xt[:, r:r + 7:2, 0:2 * OH:2]
                                nc.tensor.matmul(ps[:, :, :], lhsT=wt0[:, d, :],
                                                 rhs=rhs, start=first,
                                                 stop=(d == 3 and ti == 2))
                                first = False
                        act = stp2.tile([64, 4, OH], F32, tag="act", name="act")
                        nc.scalar.activation(act[:, :, :], ps[:, :, :], AF.Relu,
                                             bias=sb_ap, scale=ss_ap)
                        r0 = o0 + c0 * 4
                        nc.vector.tensor_max(m1[:, r0:r0 + 4, :],
                                             act[:, :, 0:112:2], act[:, :, 1:112:2])
                        nc.vector.tensor_max(m1[:, r0:r0 + 4, 0:55],
                                             m1[:, r0:r0 + 4, 0:55],
                                             act[:, :, 2:112:2])
                ts = t1[:, 0, img]
                nc.vector.tensor_max(ts[:, :, :], m1[:, 0:112:2, :], m1[:, 1:112:2, :])
                nc.vector.tensor_max(ts[:, 0:55, :], ts[:, 0:55, :], m1[:, 2:112:2, :])
                nc.scalar.activation(s1[:, 0, img, 1:57, 1:57], ts[:, :, :],
                                     AF.Sign, bias=cvec('B_stem', rows=64))

        # ================= blocks =================
        for i in range(16):
            ci, co, stride, down = BLOCKS[i]
            si, so = sec_of(i), sec_of(i + 1)
            open_sec(so)
            Hi, Ho = H_IN[i], H_IN[i] // stride
            Gi, Go = max(ci // 128, 1), max(co // 128, 1)
            K, M = min(ci, 128), min(co, 128)
            s_in = get_s(si, i % 2)
            t_in = t_tiles[si]
            t_out = t_tiles[so]
            s_out = get_s(so, (i + 1) % 2) if i < 15 else None

            for go in range(Go):
                wt = w_pool.tile([K, 9, Gi, M], F8, tag="wb", name="wb")
                src = bass.AP(tensor=io[f'wb{i}'].tensor, offset=go * M,
                              ap=[[co, K], [ci * co, 9], [128 * co, Gi], [1, M]])
                nc.sync.dma_start(out=wt[:, :, :, :], in_=src)
                sb_i = cvec(f'sb{i}', go, rows=M)
                B_i = cvec(f'B{i}', go, rows=M) if i < 15 else None
                allc = _chunks(Ho)
                CG = 1
                for cg0 in range(0, len(allc), CG):
                    grp = allc[cg0:cg0 + CG]
                    pss, psfs = [], []
                    for (img, r0, r1) in grp:
                        nr = r1 - r0
                        pshape = [M, NB, nr, Ho] if img is None else [M, nr, Ho]
                        ps = psum_pool.tile(pshape, F32, tag=f"ps{len(pss)}",
                                            name="ps")
                        pss.append(ps)
                        psfs.append(ps[:, :, :, :] if img is None else ps[:, :, :])
                    k_mm = 0
                    for gi in range(Gi):
                        for tap in range(9):
                            kh, kw = tap // 3, tap % 3
                            k_mm += 1
                            for ci_, (img, r0, r1) in enumerate(grp):
                                if stride == 1:
                                    rhs = (s_in[:, gi, :, r0 + kh:r1 + kh, kw:kw + Ho]
                                           if img is None else
                                           s_in[:, gi, img, r0 + kh:r1 + kh, kw:kw + Ho])
                                else:
                                    rhs = (s_in[:, gi, :, 2 * r0 + kh + 1:2 * r1 + kh:2,
                                                kw + 1:kw + 2 * Ho:2]
                                           if img is None else
                                           s_in[:, gi, img, 2 * r0 + kh + 1:2 * r1 + kh:2,
                                                kw + 1:kw + 2 * Ho:2])
                                nc.tensor.matmul(psfs[ci_], lhsT=wt[:, tap, gi, :],
                                                 rhs=rhs, start=(k_mm == 1),
                                                 stop=(k_mm == Gi * 9))
                    for ci_, (img, r0, r1) in enumerate(grp):
                        nr = r1 - r0
                        ps, psf = pss[ci_], psfs[ci_]

                        pshape = [M, NB, nr, Ho] if img is None else [M, nr, Ho]

                        def tsl(tt, g):
                            return (tt[:, g, :, r0:r1, :] if img is None
                                    else tt[:, g, img, r0:r1, :])

                        if not down:
                            tin = tsl(t_in, go)
                            nc.vector.scalar_tensor_tensor(
                                out=tin, in0=psf, scalar=sb_i, in1=tin,
                                op0=ALU.mult, op1=ALU.add)
                            tview = tin
                        else:
                            ps2 = psum_pool.tile(pshape, F32, tag="psd", name="ps2")
                            ps2f = ps2[:, :, :, :] if img is None else ps2[:, :, :]
                            f2 = True
                            for gi in range(Gi):
                                for (dh, dw) in ((0, 0), (0, 1), (1, 0), (1, 1)):
                                    rhs = (t_in[:, gi, :, 2 * r0 + dh:2 * r1 + dh - 1:2,
                                                dw:dw + 2 * Ho - 1:2]
                                           if img is None else
                                           t_in[:, gi, img, 2 * r0 + dh:2 * r1 + dh - 1:2,
                                                dw:dw + 2 * Ho - 1:2])
                                    nc.tensor.matmul(
                                        ps2f, lhsT=wdt[i][:, gi, go * M:(go + 1) * M],
                                        rhs=rhs, start=f2,
                                        stop=(gi == Gi - 1 and (dh, dw) == (1, 1)))
                                    f2 = False
                            tmp = d_pool.tile(pshape, F32, tag="dtmp", name="dtmp")
                            tmpf = tmp[:, :, :, :] if img is None else tmp[:, :, :]
                            nc.vector.tensor_scalar_mul(tmpf, ps2f, cvec(f'sd{i}', go, rows=M))
                            tout = tsl(t_out, go)
                            nc.vector.scalar_tensor_tensor(
                                out=tout, in0=psf, scalar=sb_i, in1=tmpf,
                                op0=ALU.mult, op1=ALU.add)
                            tview = tout
                        if i < 15:
                            ssl = (s_out[:, go, :, 1 + r0:1 + r1, 1:1 + Ho] if img is None
                                   else s_out[:, go, img, 1 + r0:1 + r1, 1:1 + Ho])
                            nc.scalar.activation(ssl, tview, AF.Sign, bias=B_i)

        # ================= head =================
        h_cm = tc.tile_pool(name="head", bufs=1)
        g_pool = h_cm.__enter__()
        hw_t = g_pool.tile([128, 4, 1000], F32)
        nc.sync.dma_start(out=hw_t[:, :, :],
                          in_=bass.AP(tensor=io['whead'].tensor, offset=0,
                                      ap=[[1000, 128], [128 * 1000, 4], [1, 1000]]))
        hb_t = g_pool.tile([NB, 1000], F32)
        nc.sync.dma_start(out=hb_t[:, :],
                          in_=bass.AP(tensor=io['bhead'].tensor, offset=0,
                                      ap=[[0, NB], [1, 1000]]))
        t4 = t_tiles[4]
        u = g_pool.tile([128, 4, NB, 7, 7], F32)
        g_t = g_pool.tile([128, 4, NB], F32)
        for g in range(4):
            nc.scalar.activation(u[:, g, :, :, :], t4[:, g, :, :, :], AF.Relu,
                                 bias=cvec('B15', g))
            nc.vector.tensor_reduce(g_t[:, g, :], u[:, g, :, :, :],
                                    axis=AX.XY, op=ALU.add)
        lg = g_pool.tile([NB, 1000], F32)
        for nt in range(2):
            lps = psum_pool.tile([NB, 500], F32, tag=f"ps{nt}", name="lps")
            for g in range(4):
                nc.tensor.matmul(lps[:, :], lhsT=g_t[:, g, :],
                                 rhs=hw_t[:, g, nt * 500:(nt + 1) * 500],
                                 start=(g == 0), stop=(g == 3))
            nc.vector.scalar_tensor_tensor(
                out=lg[:, nt * 500:(nt + 1) * 500], in0=lps[:, :],
                scalar=1.0 / 49.0, in1=hb_t[:, nt * 500:(nt + 1) * 500],
                op0=ALU.mult, op1=ALU.add)
        mx = g_pool.tile([NB, 1], F32)
        nc.vector.tensor_reduce(mx[:, :], lg[:, :], axis=AX.X, op=ALU.max)
        nmx = g_pool.tile([NB, 1], F32)
        nc.vector.tensor_scalar_mul(nmx[:, :], mx[:, :], -1.0)
        e = g_pool.tile([NB, 1000], F32)
        sm = g_pool.tile([NB, 1], F32)
        nc.scalar.activation(e[:, :], lg[:, :], AF.Exp, bias=nmx[:, :],
                             accum_out=sm[:, :])
        rs = g_pool.tile([NB, 1], F32)
        nc.vector.reciprocal(rs[:, :], sm[:, :])
        o = g_pool.tile([NB, 1000], F32)
        nc.vector.tensor_scalar_mul(o[:, :], e[:, :], rs[:, :])
        nc.sync.dma_start(out=io['out'][:, :], in_=o[:, :])
        h_cm.__exit__(None, None, None)
        for s in (4, 3, 2, 1):
            close_sec(s)


# ---------------------------------------------------------------------------
# build + run
# ---------------------------------------------------------------------------

_CACHE = {}


def declare_io(nc, nc_shape):
    io = {}
    io['x_hi'] = nc.dram_tensor("x_hi", [NB, 3, HP, WP], BF16, kind="ExternalInput").ap()
    io['x_lo'] = nc.dram_tensor("x_lo", [NB, 3, HP, WP], BF16, kind="ExternalInput").ap()
    io['wst_hi'] = nc.dram_tensor("wst_hi", [4, 42, 64], BF16, kind="ExternalInput").ap()
    io['wst_lo'] = nc.dram_tensor("wst_lo", [4, 42, 64], BF16, kind="ExternalInput").ap()
    for i in range(16):
        ci, co, _, dn = BLOCKS[i]
        io[f'wb{i}'] = nc.dram_tensor(f"wb{i}", [9, ci, co], F8, kind="ExternalInput").ap()
        if dn:
            io[f'wd{i}'] = nc.dram_tensor(f"wd{i}", [ci, co], F32, kind="ExternalInput").ap()
    io['consts'] = nc.dram_tensor("consts", list(nc_shape), F32, kind="ExternalInput").ap()
    io['whead'] = nc.dram_tensor("whead", [512, 1000], F32, kind="ExternalInput").ap()
    io['bhead'] = nc.dram_tensor("bhead", [1000], F32, kind="ExternalInput").ap()
    io['out'] = nc.dram_tensor("out", [NB, 1000], F32, kind="ExternalOutput").ap()
    return io


def build(consts_shape, colmap):
    nc = bacc.Bacc("TRN2", target_bir_lowering=False, debug=False,
                   enable_asserts=False)
    io = declare_io(nc, consts_shape)
    with tile.TileContext(nc) as tc:
        emit_model(tc, io, colmap)
    nc.compile()
    return nc, io


def make_in_maps(x, pp, consts_arr):
    base = {'wst_hi': pp['wst_hi'], 'wst_lo': pp['wst_lo'], 'consts': consts_arr,
            'whead': pp['whead'], 'bhead': pp['bhead']}
    for i in range(16):
        base[f'wb{i}'] = pp[f'wb{i}']
        if f'wd{i}' in pp:
            base[f'wd{i}'] = pp[f'wd{i}']
    in_maps = []
    for c in range(NCORES):
        xh, xl = prep_x_core(np.asarray(x)[c * NB:(c + 1) * NB])
        in_maps.append({**base, 'x_hi': xh, 'x_lo': xl})
    return in_maps


def kernel(x, params):
    x = np.asarray(x, np.float32)
    pp = prep_params(params)
    consts_arr, colmap = _pack_consts(pp)
    if 'nc' not in _CACHE:
        _CACHE['nc'] = build(consts_arr.shape, colmap)
    nc, io = _CACHE['nc']
    res = run_bass_kernel_spmd(nc, make_in_maps(x, pp, consts_arr),
                               list(range(NCORES)))
    return np.concatenate([r['out'] for r in res.results], axis=0)


# revision 14
# speedup vs baseline: 1.4358x; 1.4157x over previous
"""BinaryResNetE18 forward on 8 TRN2 NeuronCores (pure data parallel).

- 32 images -> 8 cores x 4 images, no collectives.
- Device computes in "t-space": t = h - B (B = accumulated per-channel
  constant): BN+residual = one DVE op, next-block sign = one ACT op.
- Binary convs exact in bf16 (sign x sign, fp32 PSUM).
- Real-valued path (stem, shortcuts, BN, head) fp32-exact; stem conv uses
  bf16 hi/lo 3-term split (error ~2^-18; the net is chaotic so bf16 on the
  real path is catastrophic, but 2^-18 is below the flip threshold).
- Stem input: host passes x as zero-padded CHW bf16 hi/lo planes (pure
  layout/dtype transform).  Device replicates rows into a 42-partition
  (c,kw,kh-parity) tap tensor with contiguous-run DMAs; 4 kh-rounds x 3
  terms of K=42 matmuls accumulate in PSUM.
"""

import contextlib
import numpy as np
import ml_dtypes

import concourse.bass as bass
import concourse.mybir as mybir
import concourse.tile as tile
from concourse import bacc
from concourse.bass_utils import run_bass_kernel_spmd

F32 = mybir.dt.float32
BF16 = mybir.dt.bfloat16
F8 = mybir.dt.float8e4
AF = mybir.ActivationFunctionType
ALU = mybir.AluOpType
AX = mybir.AxisListType

EPS = 1e-5
NB = 4
NCORES = 8
HP, WP = 231, 236      # padded CHW x: rows -2..228, cols -2..233
OH = 112

BLOCKS = []
_c = 64
for _f in [64, 128, 256, 512]:
    for _ in range(4):
        BLOCKS.append((_c, _f, 2 if _c != _f else 1, _c != _f))
        _c = _f
H_IN = []
_h = 56
for (_ci, _co, _s, _dn) in BLOCKS:
    H_IN.append(_h)
    if _s == 2:
        _h //= 2

SEC_GEOM = {1: (64, 1, 56), 2: (128, 1, 28), 3: (128, 2, 14), 4: (128, 4, 7)}


def sec_of(i):
    """Section of block i's INPUT grid (block 16 == head)."""
    return 1 + sum(i > j for j in (4, 8, 12))


# ---------------------------------------------------------------------------
# host-side folding
# ---------------------------------------------------------------------------

def _bn_fold(p):
    s = np.float64(np.asarray(p['gamma'])) / np.sqrt(np.float64(np.asarray(p['var'])) + EPS)
    b = np.float64(np.asarray(p['beta'])) - np.float64(np.asarray(p['mean'])) * s
    return s, b


def _hi_lo(a):
    hi = np.asarray(a, np.float32).astype(ml_dtypes.bfloat16)
    lo = (np.asarray(a, np.float32) - hi.astype(np.float32)).astype(ml_dtypes.bfloat16)
    return hi, lo


def prep_params(params):
    out = {}
    s1, b1 = _bn_fold(params['stem_bn1'])
    s2, b2 = _bn_fold(params['stem_bn2'])
    out['stem_scale'] = (s1 * s2).astype(np.float32)
    out['stem_bias'] = (b1 * s2).astype(np.float32)
    wst = np.asarray(params['stem_w'], np.float32)        # [7,7,3,64]
    w42 = np.zeros((4, 42, 64), np.float32)
    for d in range(4):
        for c in range(3):
            for kw in range(7):
                for kap in range(2):
                    kh = 2 * d + kap
                    if kh < 7:
                        w42[d, c * 14 + kap * 7 + kw] = wst[kh, kw, c]
    out['wst_hi'], out['wst_lo'] = _hi_lo(w42)
    B = b2.copy()
    out['B_stem'] = B.astype(np.float32)
    for i, bp in enumerate(params['blocks']):
        ci, co, stride, down = BLOCKS[i]
        w = np.asarray(bp['w'], np.float32)
        out[f'wb{i}'] = np.ascontiguousarray(
            np.where(w >= 0, np.float32(1), np.float32(-1))
            .reshape(9, ci, co)).astype(np.dtype(ml_dtypes.float8_e4m3))
        sb, bb = _bn_fold(bp['bn'])
        if down:
            wd = np.asarray(bp['down_w'], np.float32)[0, 0]
            sd, bd = _bn_fold(bp['down_bn'])
            cd = B @ np.float64(wd)
            out[f'wd{i}'] = (wd / 4.0).astype(np.float32)
            out[f'sd{i}'] = sd.astype(np.float32)
            B = bb + cd * sd + bd
        else:
            B = B + bb
        out[f'sb{i}'] = sb.astype(np.float32)
        out[f'B{i}'] = B.astype(np.float32)
    out['whead'] = np.ascontiguousarray(np.asarray(params['head_w'], np.float32))
    out['bhead'] = np.asarray(params['head_b'], np.float32)
    return out


def _pack_consts(pp):
    cols, colmap = [], {}

    def add(name, vec):
        v = np.asarray(vec, np.float32).reshape(-1)
        g = int(np.ceil(len(v) / 128))
        colmap[name] = len(cols)
        for j in range(g):
            c = np.zeros(128, np.float32)
            ch = v[j * 128:(j + 1) * 128]
            c[:len(ch)] = ch
            cols.append(c)

    add('stem_scale', pp['stem_scale'])
    add('stem_bias', pp['stem_bias'])
    add('B_stem', pp['B_stem'])
    for i in range(16):
        add(f'sb{i}', pp[f'sb{i}'])
        add(f'B{i}', pp[f'B{i}'])
        if f'sd{i}' in pp:
            add(f'sd{i}', pp[f'sd{i}'])
    return np.ascontiguousarray(np.stack(cols, axis=1)), colmap


def prep_x_core(xc):
    xt = np.transpose(np.asarray(xc, np.float32), (0, 3, 1, 2))
    xp = np.zeros((NB, 3, HP, WP), np.float32)
    xp[:, :, 2:226, 2:226] = xt
    return _hi_lo(xp)


# ---------------------------------------------------------------------------
# device body
# ---------------------------------------------------------------------------

def _chunks(Ho):
    if Ho == 56:
        return [(i, r, r + 8) for i in range(NB) for r in range(0, 56, 8)]
    if Ho == 28:
        return [(i, r, min(r + 16, 28)) for i in range(NB) for r in range(0, 28, 16)]
    if Ho == 14:
        return [(None, 0, 7), (None, 7, 14)]
    if Ho == 7:
        return [(None, 0, 7)]
    raise ValueError(Ho)


def emit_model(tc, io, colmap):
    nc = tc.nc

    with contextlib.ExitStack() as top:
        g_pool = top.enter_context(tc.tile_pool(name="g", bufs=1))
        psum_pool = top.enter_context(tc.tile_pool(name="psum", bufs=2, space="PSUM"))
        w_pool = top.enter_context(tc.tile_pool(name="w", bufs=2))
        d_pool = top.enter_context(tc.tile_pool(name="d", bufs=2))

        NC = io['consts'].shape[1]
        ct = g_pool.tile([128, NC], F32)
        nc.sync.dma_start(out=ct[:, :], in_=io['consts'][:, :])

        def cvec(name, g=0, rows=128):
            return ct[:rows, colmap[name] + g:colmap[name] + g + 1]

        wst_hi = g_pool.tile([42, 4, 64], BF16)
        wst_lo = g_pool.tile([42, 4, 64], BF16)
        for t, nm in ((wst_hi, 'wst_hi'), (wst_lo, 'wst_lo')):
            src = bass.AP(tensor=io[nm].tensor, offset=0,
                          ap=[[64, 42], [42 * 64, 4], [1, 64]])
            nc.sync.dma_start(out=t[:, :, :], in_=src)

        wdt = {}
        for i in (4, 8, 12):
            ci, co, _, _ = BLOCKS[i]
            gi, k = max(ci // 128, 1), min(ci, 128)
            t = g_pool.tile([k, gi, co], F32, name=f"wd{i}_t")
            src = bass.AP(tensor=io[f'wd{i}'].tensor, offset=0,
                          ap=[[co, k], [128 * co, gi], [1, co]])
            nc.sync.dma_start(out=t[:, :, :], in_=src)
            wdt[i] = t

        # section pools opened lazily, closed when dead
        sec_cm, sec_pool, t_tiles, s_tiles = {}, {}, {}, {}

        def open_sec(s):
            if s in sec_pool:
                return
            sec_cm[s] = tc.tile_pool(name=f"sec{s}", bufs=1)
            p = sec_cm[s].__enter__()
            sec_pool[s] = p
            P, G, H = SEC_GEOM[s]
            t_tiles[s] = p.tile([P, G, NB, H, H], F32, tag=f"t{s}", name=f"t{s}")
            s_tiles[s] = {}

        def get_s(s, k):
            if k not in s_tiles[s]:
                P, G, H = SEC_GEOM[s]
                sp = sec_pool[s].tile([P, G, NB, H + 2, H + 2], F8,
                                      tag=f"s{s}_{k}", name=f"s{s}_{k}")
                nc.vector.memset(sp[:, :, :, :, :], 0.0)
                s_tiles[s][k] = sp
            return s_tiles[s][k]

        def close_sec(s):
            sec_cm[s].__exit__(None, None, None)

        # ================= STEM =================
        open_sec(1)
        t1, s1 = t_tiles[1], get_s(1, 0)
        with tc.tile_pool(name="stem", bufs=1) as stp, \
                tc.tile_pool(name="stm2", bufs=2) as stp2:
            sb_ap = cvec('stem_bias', rows=64)
            ss_ap = cvec('stem_scale', rows=64)
            for img in range(NB):
                m1 = stp.tile([64, 112, 56], F32, tag="m1", name="m1", bufs=2)
                for slab in range(14):
                    o0 = slab * 8
                    xwh = stp.tile([42, 21, WP], BF16, tag="xwh", name="xwh", bufs=2)
                    xwl = stp.tile([42, 21, WP], BF16, tag="xwl", name="xwl", bufs=2)
                    for xt, nm in ((xwh, 'x_hi'), (xwl, 'x_lo')):
                        for c in range(3):
                            for kap in range(2):
                                sap = bass.AP(
                                    tensor=io[nm].tensor,
                                    offset=(img * 3 + c) * HP * WP
                                    + (2 * o0 + kap) * WP,
                                    ap=[[1, 7], [1, 21 * WP]])
                                nc.sync.dma_start(
                                    out=xt[c * 14 + kap * 7:c * 14 + kap * 7 + 7, :, :],
                                    in_=sap)
                    for c0 in range(2):
                        ps = psum_pool.tile([64, 4, OH], F32, tag=f"ps{c0}", name="ps")
                        first = True
                        for d in range(4):
                            r = 8 * c0 + 2 * d
                            for ti, (xt, wt0) in enumerate(
                                    ((xwh, wst_hi), (xwh, wst_lo), (xwl, wst_hi))):
                                rhs = xt[:, r:r + 7:2, 0:2 * OH:2]
                                nc.tensor.matmul(ps[:, :, :], lhsT=wt0[:, d, :],
                                                 rhs=rhs, start=first,
                                                 stop=(d == 3 and ti == 2))
                                first = False
                        act = stp2.tile([64, 4, OH], F32, tag="act", name="act")
                        nc.scalar.activation(act[:, :, :], ps[:, :, :], AF.Relu,
                                             bias=sb_ap, scale=ss_ap)
                        r0 = o0 + c0 * 4
                        nc.vector.tensor_max(m1[:, r0:r0 + 4, :],
                                             act[:, :, 0:112:2], act[:, :, 1:112:2])
                        nc.vector.tensor_max(m1[:, r0:r0 + 4, 0:55],
                                             m1[:, r0:r0 + 4, 0:55],
                                             act[:, :, 2:112:2])
                ts = t1[:, 0, img]
                nc.vector.tensor_max(ts[:, :, :], m1[:, 0:112:2, :], m1[:, 1:112:2, :])
                nc.vector.tensor_max(ts[:, 0:55, :], ts[:, 0:55, :], m1[:, 2:112:2, :])
                nc.scalar.activation(s1[:, 0, img, 1:57, 1:57], ts[:, :, :],
                                     AF.Sign, bias=cvec('B_stem', rows=64))

        # ================= blocks =================
        for i in range(16):
            ci, co, stride, down = BLOCKS[i]
            si, so = sec_of(i), sec_of(i + 1)
            open_sec(so)
            Hi, Ho = H_IN[i], H_IN[i] // stride
            Gi, Go = max(ci // 128, 1), max(co // 128, 1)
            K, M = min(ci, 128), min(co, 128)
            s_in = get_s(si, i % 2)
            t_in = t_tiles[si]
            t_out = t_tiles[so]
            s_out = get_s(so, (i + 1) % 2) if i < 15 else None

            for go in range(Go):
                wt = w_pool.tile([K, 9, Gi, M], F8, tag="wb", name="wb")
                src = bass.AP(tensor=io[f'wb{i}'].tensor, offset=go * M,
                              ap=[[co, K], [ci * co, 9], [128 * co, Gi], [1, M]])
                nc.sync.dma_start(out=wt[:, :, :, :], in_=src)
                sb_i = cvec(f'sb{i}', go, rows=M)
                B_i = cvec(f'B{i}', go, rows=M) if i < 15 else None
                allc = _chunks(Ho)
                CG = 1
                for cg0 in range(0, len(allc), CG):
                    grp = allc[cg0:cg0 + CG]
                    pss, psfs = [], []
                    for (img, r0, r1) in grp:
                        nr = r1 - r0
                        pshape = [M, NB, nr, Ho] if img is None else [M, nr, Ho]
                        ps = psum_pool.tile(pshape, F32, tag=f"ps{len(pss)}",
                                            name="ps")
                        pss.append(ps)
                        psfs.append(ps[:, :, :, :] if img is None else ps[:, :, :])
                    k_mm = 0
                    for gi in range(Gi):
                        for tap in range(9):
                            kh, kw = tap // 3, tap % 3
                            k_mm += 1
                            for ci_, (img, r0, r1) in enumerate(grp):
                                if stride == 1:
                                    rhs = (s_in[:, gi, :, r0 + kh:r1 + kh, kw:kw + Ho]
                                           if img is None else
                                           s_in[:, gi, img, r0 + kh:r1 + kh, kw:kw + Ho])
                                else:
                                    rhs = (s_in[:, gi, :, 2 * r0 + kh + 1:2 * r1 + kh:2,
                                                kw + 1:kw + 2 * Ho:2]
                                           if img is None else
                                           s_in[:, gi, img, 2 * r0 + kh + 1:2 * r1 + kh:2,
                                                kw + 1:kw + 2 * Ho:2])
                                nc.tensor.matmul(psfs[ci_], lhsT=wt[:, tap, gi, :],
                                                 rhs=rhs, start=(k_mm == 1),
                                                 stop=(k_mm == Gi * 9))
                    for ci_, (img, r0, r1) in enumerate(grp):
                        nr = r1 - r0
                        ps, psf = pss[ci_], psfs[ci_]

                        pshape = [M, NB, nr, Ho] if img is None else [M, nr, Ho]

                        def tsl(tt, g):
                            return (tt[:, g, :, r0:r1, :] if img is None
                                    else tt[:, g, img, r0:r1, :])

                        if not down:
                            tin = tsl(t_in, go)
                            nc.vector.scalar_tensor_tensor(
                                out=tin, in0=psf, scalar=sb_i, in1=tin,
                                op0=ALU.mult, op1=ALU.add)
                            tview = tin
                        else:
                            ps2 = psum_pool.tile(pshape, F32, tag="psd", name="ps2")
                            ps2f = ps2[:, :, :, :] if img is None else ps2[:, :, :]
                            f2 = True
                            for gi in range(Gi):
                                for (dh, dw) in ((0, 0), (0, 1), (1, 0), (1, 1)):
                                    rhs = (t_in[:, gi, :, 2 * r0 + dh:2 * r1 + dh - 1:2,
                                                dw:dw + 2 * Ho - 1:2]
                                           if img is None else
                                           t_in[:, gi, img, 2 * r0 + dh:2 * r1 + dh - 1:2,
                                                dw:dw + 2 * Ho - 1:2])
                                    nc.tensor.matmul(
                                        ps2f, lhsT=wdt[i][:, gi, go * M:(go + 1) * M],
                                        rhs=rhs, start=f2,
                                        stop=(gi == Gi - 1 and (dh, dw) == (1, 1)))
                                    f2 = False
                            tmp = d_pool.tile(pshape, F32, tag="dtmp", name="dtmp")
                            tmpf = tmp[:, :, :, :] if img is None else tmp[:, :, :]
                            nc.vector.tensor_scalar_mul(tmpf, ps2f, cvec(f'sd{i}', go, rows=M))
                            tout = tsl(t_out, go)
                            nc.vector.scalar_tensor_tensor(
                                out=tout, in0=psf, scalar=sb_i, in1=tmpf,
                                op0=ALU.mult, op1=ALU.add)
                            tview = tout
                        if i < 15:
                            ssl = (s_out[:, go, :, 1 + r0:1 + r1, 1:1 + Ho] if img is None
                                   else s_out[:, go, img, 1 + r0:1 + r1, 1:1 + Ho])
                            nc.scalar.activation(ssl, tview, AF.Sign, bias=B_i)

        # ================= head =================
        h_cm = tc.tile_pool(name="head", bufs=1)
        g_pool = h_cm.__enter__()
        hw_t = g_pool.tile([128, 4, 1000], F32)
        nc.sync.dma_start(out=hw_t[:, :, :],
                          in_=bass.AP(tensor=io['whead'].tensor, offset=0,
                                      ap=[[1000, 128], [128 * 1000, 4], [1, 1000]]))
        hb_t = g_pool.tile([NB, 1000], F32)
        nc.sync.dma_start(out=hb_t[:, :],
                          in_=bass.AP(tensor=io['bhead'].tensor, offset=0,
                                      ap=[[0, NB], [1, 1000]]))
        t4 = t_tiles[4]
        u = g_pool.tile([128, 4, NB, 7, 7], F32)
        g_t = g_pool.tile([128, 4, NB], F32)
        for g in range(4):
            nc.scalar.activation(u[:, g, :, :, :], t4[:, g, :, :, :], AF.Relu,
                                 bias=cvec('B15', g))
            nc.vector.tensor_reduce(g_t[:, g, :], u[:, g, :, :, :],
                                    axis=AX.XY, op=ALU.add)
        lg = g_pool.tile([NB, 1000], F32)
        for nt in range(2):
            lps = psum_pool.tile([NB, 500], F32, tag=f"ps{nt}", name="lps")
            for g in range(4):
                nc.tensor.matmul(lps[:, :], lhsT=g_t[:, g, :],
                                 rhs=hw_t[:, g, nt * 500:(nt + 1) * 500],
                                 start=(g == 0), stop=(g == 3))
            nc.vector.scalar_tensor_tensor(
                out=lg[:, nt * 500:(nt + 1) * 500], in0=lps[:, :],
                scalar=1.0 / 49.0, in1=hb_t[:, nt * 500:(nt + 1) * 500],
                op0=ALU.mult, op1=ALU.add)
        mx = g_pool.tile([NB, 1], F32)
        nc.vector.tensor_reduce(mx[:, :], lg[:, :], axis=AX.X, op=ALU.max)
        nmx = g_pool.tile([NB, 1], F32)
        nc.vector.tensor_scalar_mul(nmx[:, :], mx[:, :], -1.0)
        e = g_pool.tile([NB, 1000], F32)
        sm = g_pool.tile([NB, 1], F32)
        nc.scalar.activation(e[:, :], lg[:, :], AF.Exp, bias=nmx[:, :],
                             accum_out=sm[:, :])
        rs = g_pool.tile([NB, 1], F32)
        nc.vector.reciprocal(rs[:, :], sm[:, :])
        o = g_pool.tile([NB, 1000], F32)
        nc.vector.tensor_scalar_mul(o[:, :], e[:, :], rs[:, :])
        nc.sync.dma_start(out=io['out'][:, :], in_=o[:, :])
        h_cm.__exit__(None, None, None)
        for s in (4, 3, 2, 1):
            close_sec(s)


# ---------------------------------------------------------------------------
# build + run
# ---------------------------------------------------------------------------

_CACHE = {}


def declare_io(nc, nc_shape):
    io = {}
    io['x_hi'] = nc.dram_tensor("x_hi", [NB, 3, HP, WP], BF16, kind="ExternalInput").ap()
    io['x_lo'] = nc.dram_tensor("x_lo", [NB, 3, HP, WP], BF16, kind="ExternalInput").ap()
    io['wst_hi'] = nc.dram_tensor("wst_hi", [4, 42, 64], BF16, kind="ExternalInput").ap()
    io['wst_lo'] = nc.dram_tensor("wst_lo", [4, 42, 64], BF16, kind="ExternalInput").ap()
    for i in range(16):
        ci, co, _, dn = BLOCKS[i]
        io[f'wb{i}'] = nc.dram_tensor(f"wb{i}", [9, ci, co], F8, kind="ExternalInput").ap()
        if dn:
            io[f'wd{i}'] = nc.dram_tensor(f"wd{i}", [ci, co], F32, kind="ExternalInput").ap()
    io['consts'] = nc.dram_tensor("consts", list(nc_shape), F32, kind="ExternalInput").ap()
    io['whead'] = nc.dram_tensor("whead", [512, 1000], F32, kind="ExternalInput").ap()
    io['bhead'] = nc.dram_tensor("bhead", [1000], F32, kind="ExternalInput").ap()
    io['out'] = nc.dram_tensor("out", [NB, 1000], F32, kind="ExternalOutput").ap()
    return io


def build(consts_shape, colmap):
    nc = bacc.Bacc("TRN2", target_bir_lowering=False, debug=False,
                   enable_asserts=False)
    io = declare_io(nc, consts_shape)
    with tile.TileContext(nc) as tc:
        emit_model(tc, io, colmap)
    nc.compile()
    return nc, io


def make_in_maps(x, pp, consts_arr):
    base = {'wst_hi': pp['wst_hi'], 'wst_lo': pp['wst_lo'], 'consts': consts_arr,
            'whead': pp['whead'], 'bhead': pp['bhead']}
    for i in range(16):
        base[f'wb{i}'] = pp[f'wb{i}']
        if f'wd{i}' in pp:
            base[f'wd{i}'] = pp[f'wd{i}']
    in_maps = []
    for c in range(NCORES):
        xh, xl = prep_x_core(np.asarray(x)[c * NB:(c + 1) * NB])
        in_maps.append({**base, 'x_hi': xh, 'x_lo': xl})
    return in_maps


def kernel(x, params):
    x = np.asarray(x, np.float32)
    pp = prep_params(params)
    consts_arr, colmap = _pack_consts(pp)
    if 'nc' not in _CACHE:
        _CACHE['nc'] = build(consts_arr.shape, colmap)
    nc, io = _CACHE['nc']
    res = run_bass_kernel_spmd(nc, make_in_maps(x, pp, consts_arr),
                               list(range(NCORES)))
    return np.concatenate([r['out'] for r in res.results], axis=0)
